# revision 1
# baseline (speedup 1.0000x reference)
"""Trainium2 Bass kernel for nn_Attention_31267361915369 (v3).

Computation (per batch example, T=4096, D=1024):
    h   = tanh(x @ W1.T + b1)          # [T, D]
    s   = h @ w2.T + b2                # [T]
    e   = exp(s)                       # no max-subtraction: |s| <= sum|w2| ~ 26,
                                       # and num/den is exactly shift-invariant
    num = cumsum(e * x, axis=0)        # [T, D]
    den = cumsum(e)                    # [T]
    out = tanh([num/den, x] @ Wc.T + bc)

Restructure (v2): cumsum over T commutes with the D-side matmul, so with
A = Wc[:, :D], B = Wc[:, D:]:
    out = tanh(cumsum(e * y)/den + z),  y = x @ A.T, z = x @ B.T
All three [D,D] matmuls (24 k-passes of 1024 cols/tile) consume one
host-pretransposed xT tile back-to-back on TensorE; no ctx transpose.

v3 scheduling fixes on top:
  - DMA queues are per-issuing-engine (q<eng>DynamicHW). xT loads stay on
    SP (a loads-only queue, prefetched one tile ahead); output + carry-row
    stores issue from the ACT engine's queue so loads never serialize
    behind stores.
  - stage-skewed emission: tile i-1's combine (ctx = pc*rden, +z, tanh)
    is emitted at the top of iteration i so tanh_out(i-1) sits before
    tanh(i) in the ACT queue at the moment each becomes ready.
  - only hw-proven DVE/ACT op classes (scalar_tensor_tensor and
    tensor_tensor_reduce both crash/hang this hw's DVE).

PSUM (8 banks): one 3-buffer pool of [128,1024] f32 hosts ph/py/pz/pc in
rotation (6 banks) + a 2-buffer [128,1] pool for the den cumsum (2).

Distribution: data-parallel over batch B=8 across 8 NeuronCores, weights
replicated, no collectives.
"""

import sys

if "/opt/trn_rl_repo" not in sys.path:
    sys.path.insert(0, "/opt/trn_rl_repo")

from contextlib import ExitStack

import ml_dtypes
import numpy as np

import concourse.bass as bass
import concourse.tile as tile
from concourse import bacc, mybir
from concourse.bass_utils import run_bass_kernel_spmd

P = 128
D = 1024
T_FULL = 4096
N_CORES = 8
NK = D // P

BF = mybir.dt.bfloat16
F32 = mybir.dt.float32
AFT = mybir.ActivationFunctionType
ALU = mybir.AluOpType

_BUILD_CACHE: dict = {}


def build(T: int = T_FULL, use_b1: bool = False, use_bc: bool = False,
          repeat: int = 1):
    key = (T, use_b1, use_bc, repeat)
    if key in _BUILD_CACHE:
        return _BUILD_CACHE[key]

    assert T % P == 0
    NT = T // P

    nc = bacc.Bacc("TRN2", target_bir_lowering=False, debug=False)

    # host-pretransposed x: xt[i, p, k, t] = x[i*128+t, k*128+p]
    xt_d = nc.declare_dram_parameter("xt", [T * NK, P], BF, isOutput=False)
    w1t_d = nc.declare_dram_parameter("w1t", [D, D], BF, isOutput=False)
    wat_d = nc.declare_dram_parameter("wat", [D, D], BF, isOutput=False)
    wbt_d = nc.declare_dram_parameter("wbt", [D, D], BF, isOutput=False)
    w2r_d = nc.declare_dram_parameter("w2r", [P, D], F32, isOutput=False)
    tri_d = nc.declare_dram_parameter("tri", [P, P], BF, isOutput=False)
    b1_d = nc.declare_dram_parameter("b1r", [1, D], BF, isOutput=False) if use_b1 else None
    bc_d = nc.declare_dram_parameter("bcr", [1, D], BF, isOutput=False) if use_bc else None
    out_d = nc.declare_dram_parameter("out", [T, D], F32, isOutput=True)

    xt_t = xt_d.ap().rearrange("(n p k) q -> n p k q", p=P, k=NK)
    out_t = out_d.ap().rearrange("(n p) d -> n p d", p=P)
    w1_t = w1t_d.ap().rearrange("(k p) e -> k p e", p=P)
    wa_t = wat_d.ap().rearrange("(k p) e -> k p e", p=P)
    wb_t = wbt_d.ap().rearrange("(k p) e -> k p e", p=P)

    with tile.TileContext(nc) as tc, ExitStack() as ctx:
        consts = ctx.enter_context(tc.tile_pool(name="consts", bufs=1))
        xtp = ctx.enter_context(tc.tile_pool(name="xtp", bufs=4))
        hpool = ctx.enter_context(tc.tile_pool(name="hpool", bufs=2))
        scr = ctx.enter_context(tc.tile_pool(name="scr", bufs=2))
        eyp = ctx.enter_context(tc.tile_pool(name="eyp", bufs=2))
        zp = ctx.enter_context(tc.tile_pool(name="zp", bufs=2))
        u1p = ctx.enter_context(tc.tile_pool(name="u1p", bufs=2))
        up = ctx.enter_context(tc.tile_pool(name="up", bufs=2))
        outp = ctx.enter_context(tc.tile_pool(name="outp", bufs=2))
        colp = ctx.enter_context(tc.tile_pool(name="colp", bufs=4))
        cstp = ctx.enter_context(tc.tile_pool(name="cstp", bufs=2))
        crowp = ctx.enter_context(tc.tile_pool(name="crowp", bufs=2))
        pmm = ctx.enter_context(tc.tile_pool(name="pmm", bufs=3, space="PSUM"))
        ppd = ctx.enter_context(tc.tile_pool(name="ppd", bufs=2, space="PSUM"))

        tri_sb = consts.tile([P, P], BF, tag="tri")
        nc.sync.dma_start(out=tri_sb[:], in_=tri_d.ap())
        # f32: wide bf16 TensorTensor/TensorReduce DVE ops hang on this hw
        w2r_sb = consts.tile([P, D], F32, tag="w2r")
        nc.sync.dma_start(out=w2r_sb[:], in_=w2r_d.ap())
        if use_b1:
            b1_sb = consts.tile([1, D], BF, tag="b1")
            nc.sync.dma_start(out=b1_sb[:], in_=b1_d.ap())
        if use_bc:
            bc_sb = consts.tile([1, D], BF, tag="bc")
            nc.sync.dma_start(out=bc_sb[:], in_=bc_d.ap())
        w1_sb, wa_sb, wb_sb = [], [], []
        for k in range(NK):
            t1 = consts.tile([P, D], BF, tag=f"w1_{k}")
            nc.sync.dma_start(out=t1[:], in_=w1_t[k])
            w1_sb.append(t1)
        for k in range(NK):
            ta = consts.tile([P, D], BF, tag=f"wa_{k}")
            nc.sync.dma_start(out=ta[:], in_=wa_t[k])
            wa_sb.append(ta)
        for k in range(NK):
            tb = consts.tile([P, D], BF, tag=f"wb_{k}")
            nc.sync.dma_start(out=tb[:], in_=wb_t[k])
            wb_sb.append(tb)

        state = {}

        def load_xt(i):
            xT = xtp.tile([P, NK, P], BF, tag="xt")
            nc.sync.dma_start(out=xT[:], in_=xt_t[i])
            state[("xt", i)] = xT

        def combine(i):
            """ctx = pc*rden; u = ctx + z; out = tanh(u); store."""
            pc, rden, z_sb = state.pop(("fin", i))
            u1 = u1p.tile([P, D], F32, tag="u1")
            nc.vector.tensor_scalar_mul(u1[:], pc[:], rden[:])
            u_sb = up.tile([P, D], F32, tag="u")
            nc.vector.tensor_add(u_sb[:], u1[:], z_sb[:])
            o_sb = outp.tile([P, D], F32, tag="out")
            nc.scalar.activation(o_sb[:], u_sb[:], AFT.Tanh)
            nc.sync.dma_start(out=out_t[i], in_=o_sb[:])

        def front(i):
            xT = state.pop(("xt", i))

            ph = pmm.tile([P, D], F32, tag="mm", name="ph")
            for k in range(NK):
                last = k == NK - 1 and not use_b1
                for c in range(2):
                    nc.tensor.matmul(
                        ph[:, c * 512:(c + 1) * 512],
                        xT[:, k, :],
                        w1_sb[k][:, c * 512:(c + 1) * 512],
                        start=(k == 0),
                        stop=last,
                    )
            if use_b1:
                for c in range(2):
                    nc.tensor.matmul(
                        ph[:, c * 512:(c + 1) * 512],
                        tri_sb[0:1, :],
                        b1_sb[0:1, c * 512:(c + 1) * 512],
                        start=False,
                        stop=True,
                    )
            h_sb = hpool.tile([P, D], F32, tag="h")
            nc.scalar.activation(h_sb[:], ph[:], AFT.Tanh)

            prod = scr.tile([P, D], F32, tag="prod")
            s_col = colp.tile([P, 1], F32, tag="s")
            nc.vector.tensor_mul(prod[:], h_sb[:], w2r_sb[:])
            nc.vector.reduce_sum(s_col[:], prod[:], axis=mybir.AxisListType.X)

            e_col = colp.tile([P, 1], F32, tag="e")
            nc.scalar.activation(e_col[:], s_col[:], AFT.Exp)
            ey = eyp.tile([P, D + 1], BF, tag="ey")
            nc.scalar.copy(ey[:, D:D + 1], e_col[:])

            py = pmm.tile([P, D], F32, tag="mm", name="py")
            for k in range(NK):
                for c in range(2):
                    nc.tensor.matmul(
                        py[:, c * 512:(c + 1) * 512],
                        xT[:, k, :],
                        wa_sb[k][:, c * 512:(c + 1) * 512],
                        start=(k == 0),
                        stop=(k == NK - 1),
                    )
            nc.vector.tensor_scalar_mul(ey[:, 0:D], py[:], e_col[:])

            pz = pmm.tile([P, D], F32, tag="mm", name="pz")
            for k in range(NK):
                last = k == NK - 1 and not use_bc
                for c in range(2):
                    nc.tensor.matmul(
                        pz[:, c * 512:(c + 1) * 512],
                        xT[:, k, :],
                        wb_sb[k][:, c * 512:(c + 1) * 512],
                        start=(k == 0),
                        stop=last,
                    )
            if use_bc:
                for c in range(2):
                    nc.tensor.matmul(
                        pz[:, c * 512:(c + 1) * 512],
                        tri_sb[0:1, :],
                        bc_sb[0:1, c * 512:(c + 1) * 512],
                        start=False,
                        stop=True,
                    )
            z_sb = zp.tile([P, D], F32, tag="z")
            nc.scalar.copy(z_sb[:], pz[:])
            state[("mid", i)] = (ey, z_sb)

        def cumsum(i):
            ey, z_sb = state.pop(("mid", i))
            # inject the running carry into ey row 0: tri row 0 is all
            # ones, so the matmul propagates it to every output row.
            if i > 0:
                crow = state.pop(("crow", i - 1))
                nc.vector.tensor_add(ey[0:1, :], ey[0:1, :], crow[0:1, :])

            pc = pmm.tile([P, D], F32, tag="mm", name="pc")
            pd = ppd.tile([P, 1], F32, tag="pd")
            nc.tensor.matmul(pd[:], tri_sb[:], ey[:, D:D + 1], start=True, stop=True)
            for c in range(2):
                nc.tensor.matmul(
                    pc[:, c * 512:(c + 1) * 512],
                    tri_sb[:],
                    ey[:, c * 512:(c + 1) * 512],
                    start=True,
                    stop=True,
                )

            rden = colp.tile([P, 1], F32, tag="rden")
            nc.vector.reciprocal(rden[:], pd[:])

            # running totals (row 127) -> bf16 -> DMA to partition 0 for
            # the next tile. engines can't cross partitions and must start
            # at a 32-aligned partition, hence the [96:128] staging copy.
            if i < NT - 1:
                cstage = cstp.tile([P, D + 1], BF, tag="cst")
                nc.scalar.copy(cstage[96:128, 0:D], pc[96:128, :])
                nc.scalar.copy(cstage[96:128, D:D + 1], pd[96:128, :])
                crow = crowp.tile([1, D + 1], BF, tag="crow")
                nc.sync.dma_start(out=crow[0:1, :], in_=cstage[127:128, :])
                state[("crow", i)] = crow

            state[("fin", i)] = (pc, rden, z_sb)

        def whole_pipeline():
            state.clear()
            load_xt(0)
            load_xt(1)
            for i in range(NT):
                if i + 2 < NT:
                    load_xt(i + 2)
                if i > 0:
                    combine(i - 1)
                front(i)
                cumsum(i)
            combine(NT - 1)

        if repeat == 1:
            whole_pipeline()
        else:
            with tc.For_i(0, repeat, 1):
                whole_pipeline()

    nc.compile()
    _BUILD_CACHE[key] = nc
    return nc


def _bf16(a):
    return np.ascontiguousarray(np.asarray(a, dtype=np.float32)).astype(
        ml_dtypes.bfloat16
    )


def make_in_maps(x, W1, b1, w2, b2, Wc, bc, T=T_FULL):
    """Host-side prep: shard x over batch, pre-transpose/replicate weights."""
    x = np.asarray(x, dtype=np.float32)
    W1 = np.asarray(W1, dtype=np.float32)
    Wc = np.asarray(Wc, dtype=np.float32)
    w2 = np.asarray(w2, dtype=np.float32).reshape(1, -1)
    b1 = np.asarray(b1, dtype=np.float32)
    bc = np.asarray(bc, dtype=np.float32)
    use_b1 = bool(np.any(b1 != 0.0))
    use_bc = bool(np.any(bc != 0.0))
    # b2 shifts every score equally; exp(b2) cancels in num/den.

    w1t = _bf16(W1.T)
    wat = _bf16(Wc[:, :D].T)
    wbt = _bf16(Wc[:, D:].T)
    w2r = np.ascontiguousarray(np.broadcast_to(w2, (P, D)).astype(np.float32))
    tri = _bf16(np.triu(np.ones((P, P), np.float32)))

    NT = T // P
    in_maps = []
    for i in range(N_CORES):
        xb = _bf16(x[i, :T, :])
        # xt[i, p, k, t] = x[i*128+t, k*128+p], 2KB-contiguous per partition
        xt = np.ascontiguousarray(
            xb.reshape(NT, P, NK, P).transpose(0, 3, 2, 1)
        ).reshape(T * NK, P)
        m = {
            "xt": xt,
            "w1t": w1t,
            "wat": wat,
            "wbt": wbt,
            "w2r": w2r,
            "tri": tri,
        }
        if use_b1:
            m["b1r"] = _bf16(b1.reshape(1, D))
        if use_bc:
            m["bcr"] = _bf16(bc.reshape(1, D))
        in_maps.append(m)
    return in_maps, use_b1, use_bc


def kernel(x, W1, b1, w2, b2, Wc, bc):
    in_maps, use_b1, use_bc = make_in_maps(x, W1, b1, w2, b2, Wc, bc)
    nc = build(T_FULL, use_b1, use_bc)
    res = run_bass_kernel_spmd(nc, in_maps, core_ids=list(range(N_CORES)))
    out = np.stack([np.asarray(res.results[i]["out"]) for i in range(N_CORES)], axis=0)
    return out.astype(np.float32)

